# revision 6
# baseline (speedup 1.0000x reference)
"""Trainium2 Bass kernel for nn_Block_41893111005254 (dense transformer block).

Reference computation (per batch b of 2, seq 2048, d 1024, 16 heads, ff 4096):
    x = x + MHSA(x)           (no 1/sqrt(hd) scaling)
    x = LN(x, g1, beta1)
    x = x + gelu(x @ W1 + b1) @ W2 + b2
    x = LN(x, g2, beta2)

Sharding: sequence-parallel, collective-free. Core c handles batch c//4,
query tokens (c%4)*512 .. +512. Each core recomputes K/V for its batch's full
sequence (redundant across the 4 cores of a batch but avoids collectives).

All on-chip activations are kept TRANSPOSED ([d, token] with d partition-
chunked) so that:
  - all matmuls consume natural-layout bf16 weights as lhsT
  - LayerNorm reductions (over d) run on the TensorEngine via ones-matmuls
  - softmax denominators come free via a ones-column appended to V
  - no transposes are needed anywhere
"""

import sys

sys.path.insert(0, "/opt/trn_rl_repo")

import numpy as np
import ml_dtypes

import concourse.bacc as bacc
import concourse.tile as tile
from concourse import mybir
from concourse.bass_utils import run_bass_kernel_spmd

P = 128
D = 1024
S = 2048
QT = 512  # query tokens per core
FF = 4096
H = 16
HD = 64
DC = D // P  # 8 chunks of d
FC = FF // P  # 32 chunks of ff
NKC = S // P  # 16 kv token chunks
LN_EPS = 1e-5

bf = mybir.dt.bfloat16
f32 = mybir.dt.float32
AF = mybir.ActivationFunctionType
OP = mybir.AluOpType

_NC_CACHE = None


def _build_nc():
    nc = bacc.Bacc(None, target_bir_lowering=False, debug=False)

    # ---- I/O declarations (per-core shards, host-prepped layouts) ----
    xT_d = nc.dram_tensor("xT", [P, DC, S], bf, kind="ExternalInput")
    xqT_d = nc.dram_tensor("xqT", [P, DC, QT], bf, kind="ExternalInput")
    xqTf_d = nc.dram_tensor("xqTf", [P, DC, QT], f32, kind="ExternalInput")
    wq_d = nc.dram_tensor("wq", [DC, P, DC, P], bf, kind="ExternalInput")
    wk_d = nc.dram_tensor("wk", [DC, P, DC, P], bf, kind="ExternalInput")
    wv_d = nc.dram_tensor("wv", [P, DC, D], bf, kind="ExternalInput")
    wo_d = nc.dram_tensor("wo", [DC, P, DC, P], bf, kind="ExternalInput")
    w1_d = nc.dram_tensor("w1", [FC, P, DC, P], bf, kind="ExternalInput")
    w2_d = nc.dram_tensor("w2", [DC, P, FC, P], bf, kind="ExternalInput")
    bq_d = nc.dram_tensor("bq", [P, DC], f32, kind="ExternalInput")
    bk_d = nc.dram_tensor("bk", [P, DC], f32, kind="ExternalInput")
    bvb_d = nc.dram_tensor("bvb", [P, D], f32, kind="ExternalInput")
    bo_d = nc.dram_tensor("bo", [P, DC], f32, kind="ExternalInput")
    b1_d = nc.dram_tensor("b1", [P, FC], f32, kind="ExternalInput")
    b2_d = nc.dram_tensor("b2", [P, DC], f32, kind="ExternalInput")
    g1_d = nc.dram_tensor("g1", [P, DC], f32, kind="ExternalInput")
    be1_d = nc.dram_tensor("be1", [P, DC], f32, kind="ExternalInput")
    g2_d = nc.dram_tensor("g2", [P, DC], f32, kind="ExternalInput")
    be2_d = nc.dram_tensor("be2", [P, DC], f32, kind="ExternalInput")
    oT_d = nc.dram_tensor("oT", [P, DC, QT], f32, kind="ExternalOutput")

    with tile.TileContext(nc) as tc:
        _emit(nc, tc, locals())
    nc.compile()
    return nc


def _emit(nc, tc, d):
    xT_d, xqT_d, xqTf_d = d["xT_d"], d["xqT_d"], d["xqTf_d"]
    wq_d, wk_d, wv_d, wo_d, w1_d, w2_d = (
        d["wq_d"], d["wk_d"], d["wv_d"], d["wo_d"], d["w1_d"], d["w2_d"],
    )
    bq_d, bk_d, bvb_d, bo_d, b1_d, b2_d = (
        d["bq_d"], d["bk_d"], d["bvb_d"], d["bo_d"], d["b1_d"], d["b2_d"],
    )
    g1_d, be1_d, g2_d, be2_d, oT_d = (
        d["g1_d"], d["be1_d"], d["g2_d"], d["be2_d"], d["oT_d"],
    )

    from contextlib import ExitStack

    with ExitStack() as ctx:
        constp = ctx.enter_context(tc.tile_pool(name="const", bufs=1))
        dramp = ctx.enter_context(tc.tile_pool(name="drb", bufs=4, space="DRAM"))

        # ---- constants ----
        bq_t = constp.tile([P, DC], f32)
        bk_t = constp.tile([P, DC], f32)
        bvb_t = constp.tile([P, D], f32)
        bo_t = constp.tile([P, DC], f32)
        b1_t = constp.tile([P, FC], f32)
        b2_t = constp.tile([P, DC], f32)
        g1_t = constp.tile([P, DC], f32)
        be1_t = constp.tile([P, DC], f32)
        g2_t = constp.tile([P, DC], f32)
        be2_t = constp.tile([P, DC], f32)
        ones_bf = constp.tile([P, 1], bf)
        eps_t = constp.tile([1, 1], f32)
        nc.sync.dma_start(out=bq_t, in_=bq_d[:, :])
        nc.sync.dma_start(out=bk_t, in_=bk_d[:, :])
        nc.sync.dma_start(out=bvb_t, in_=bvb_d[:, :])
        nc.sync.dma_start(out=bo_t, in_=bo_d[:, :])
        nc.sync.dma_start(out=b1_t, in_=b1_d[:, :])
        nc.sync.dma_start(out=b2_t, in_=b2_d[:, :])
        nc.sync.dma_start(out=g1_t, in_=g1_d[:, :])
        nc.sync.dma_start(out=be1_t, in_=be1_d[:, :])
        nc.sync.dma_start(out=g2_t, in_=g2_d[:, :])
        nc.sync.dma_start(out=be2_t, in_=be2_d[:, :])
        nc.vector.memset(ones_bf, 1.0)
        nc.vector.memset(eps_t, LN_EPS)

        # ================= persistent across A..C =================
        with tc.tile_pool(name="poolAC", bufs=1) as pAC:
            xqTf = pAC.tile([P, DC, QT], f32)
            attnT = pAC.tile([P, DC, QT], bf)
            nc.sync.dma_start(out=xqTf, in_=xqTf_d[:, :, :])

            # ================= A..B: K^T, Q^T, V =================
            with tc.tile_pool(name="poolAB", bufs=1) as pAB:
                kT = pAB.tile([P, DC, S], bf)
                qT = pAB.tile([P, DC, QT], bf)
                v65 = pAB.tile([P, NKC, H, HD + 1], bf)

                with (
                    tc.tile_pool(name="poolA", bufs=1) as pA,
                    tc.tile_pool(name="wA", bufs=3) as wA,
                    tc.tile_pool(name="psA", bufs=3, space="PSUM") as psA,
                ):
                    xT = pA.tile([P, DC, S], bf)
                    xqT = pA.tile([P, DC, QT], bf)
                    wv_sb = pA.tile([P, DC, D], bf)
                    for ci in range(DC):
                        nc.sync.dma_start(out=xT[:, ci, :], in_=xT_d[:, ci, :])
                    for ci in range(0, DC, 2):
                        nc.sync.dma_start(
                            out=xqT[:, ci : ci + 2, :], in_=xqT_d[:, ci : ci + 2, :]
                        )
                        nc.sync.dma_start(
                            out=wv_sb[:, ci : ci + 2, :], in_=wv_d[:, ci : ci + 2, :]
                        )
                    # ones columns of V (data cols overwritten by evictions;
                    # whole-tile memset avoids 1-wide strided bf16 writes)
                    nc.vector.memset(v65, 1.0)

                    # ---- K^T = (x @ Wk)^T + bk, full batch seq ----
                    for mi in range(DC):
                        wk_t = wA.tile([P, DC, P], bf, name="wk_t", tag="w")
                        nc.sync.dma_start(out=wk_t, in_=wk_d[mi, :, :, :])
                        for th in range(2):  # token halves of 1024
                            ps = psA.tile([P, 1024], f32, name="psA", tag="ps")
                            for sub in range(2):
                                t0 = th * 1024 + sub * 512
                                for kc in range(DC):
                                    nc.tensor.matmul(
                                        ps[:, sub * 512 : (sub + 1) * 512],
                                        lhsT=wk_t[:, kc, :],
                                        rhs=xT[:, kc, t0 : t0 + 512],
                                        start=(kc == 0),
                                        stop=(kc == DC - 1),
                                    )
                            nc.scalar.activation(
                                kT[:, mi, th * 1024 : (th + 1) * 1024],
                                ps,
                                AF.Identity,
                                bias=bk_t[:, mi : mi + 1],
                            )

                    # ---- Q^T = (xq @ Wq)^T + bq ----
                    for mi in range(0, DC, 2):
                        wq_t = wA.tile([P, 2, DC, P], bf, name="wq_t", tag="w")
                        nc.sync.dma_start(
                            out=wq_t,
                            in_=wq_d[mi : mi + 2, :, :, :].rearrange(
                                "a p k m -> p a k m"
                            ),
                        )
                        ps = psA.tile([P, 1024], f32, name="psAq", tag="ps")
                        for sub in range(2):
                            for kc in range(DC):
                                nc.tensor.matmul(
                                    ps[:, sub * 512 : (sub + 1) * 512],
                                    lhsT=wq_t[:, sub, kc, :],
                                    rhs=xqT[:, kc, :],
                                    start=(kc == 0),
                                    stop=(kc == DC - 1),
                                )
                            nc.scalar.activation(
                                qT[:, mi + sub, :],
                                ps[:, sub * 512 : (sub + 1) * 512],
                                AF.Identity,
                                bias=bq_t[:, mi + sub : mi + sub + 1],
                            )

                    # ---- V natural + bv, ones col kept ----
                    for tc16 in range(NKC):
                        ps = psA.tile([P, 1024], f32, name="psAv", tag="ps")
                        for nh in range(2):
                            for kc in range(DC):
                                nc.tensor.matmul(
                                    ps[:, nh * 512 : (nh + 1) * 512],
                                    lhsT=xT[:, kc, tc16 * P : (tc16 + 1) * P],
                                    rhs=wv_sb[:, kc, nh * 512 : (nh + 1) * 512],
                                    start=(kc == 0),
                                    stop=(kc == DC - 1),
                                )
                        nc.vector.tensor_add(
                            v65[:, tc16, :, 0:HD],
                            ps.rearrange("p (h x) -> p h x", x=HD),
                            bvb_t.rearrange("p (h x) -> p h x", x=HD),
                        )

                # ================= B: attention =================
                with (
                    tc.tile_pool(name="poolB", bufs=3) as pB,
                    tc.tile_pool(name="expP", bufs=4) as expP,
                    tc.tile_pool(name="psS", bufs=2, space="PSUM") as psS,
                    tc.tile_pool(name="psO", bufs=4, space="PSUM") as psO,
                ):
                    for hp in range(H // 2):
                        h0, h1 = 2 * hp, 2 * hp + 1
                        o_ps = [
                            psO.tile([HD + 1, QT], f32, name=f"o{h}", tag="o")
                            for h in (h0, h1)
                        ]
                        for kc in range(NKC):
                            s01 = psS.tile([P, 1024], f32, name="s01", tag="s")
                            for i, h in enumerate((h0, h1)):
                                p0 = 64 * (h % 2)
                                nc.tensor.matmul(
                                    s01[:, i * 512 : (i + 1) * 512],
                                    lhsT=kT[p0 : p0 + 64, hp, kc * P : (kc + 1) * P],
                                    rhs=qT[p0 : p0 + 64, hp, :],
                                    start=True,
                                    stop=True,
                                )
                            e01 = expP.tile([P, 1024], bf, name="e01", tag="e")
                            nc.scalar.activation(e01, s01, AF.Exp)
                            for i, h in enumerate((h0, h1)):
                                nc.tensor.matmul(
                                    o_ps[i],
                                    lhsT=v65[:, kc, h, :],
                                    rhs=e01[:, i * 512 : (i + 1) * 512],
                                    start=(kc == 0),
                                    stop=(kc == NKC - 1),
                                )
                        # normalize: recip of den row, DRAM-bounce broadcast
                        for i, h in enumerate((h0, h1)):
                            rr = pB.tile([HD + 1, QT], f32, name="rr", tag="rr")
                            nc.vector.reciprocal(
                                rr[HD : HD + 1, :], o_ps[i][HD : HD + 1, :]
                            )
                            bnc = dramp.tile([1, QT], f32, name="bnc", tag="bnc")
                            nc.sync.dma_start(out=bnc, in_=rr[HD : HD + 1, :])
                            rb = pB.tile([HD, QT], f32, name="rb", tag="rb")
                            nc.sync.dma_start(
                                out=rb, in_=bnc[0:1, :].to_broadcast([HD, QT])
                            )
                            if h % 2 == 0:
                                nc.vector.tensor_mul(
                                    attnT[0:HD, hp, :], o_ps[i][0:HD, :], rb
                                )
                            else:
                                tmp = pB.tile([HD, QT], bf, name="tmpo", tag="tmpo")
                                nc.vector.tensor_mul(tmp, o_ps[i][0:HD, :], rb)
                                nc.sync.dma_start(out=attnT[HD:P, hp, :], in_=tmp)

            # ================= C..D: u = LN1(x + proj), FFN, LN2 =========
            with tc.tile_pool(name="poolCD", bufs=1) as pCD:
                u_f = pCD.tile([P, DC, QT], f32)
                u_bf = pCD.tile([P, DC, QT], bf)

                with (
                    tc.tile_pool(name="poolC", bufs=1) as pC,
                    tc.tile_pool(name="wC", bufs=3) as wC,
                    tc.tile_pool(name="psC", bufs=3, space="PSUM") as psC,
                    tc.tile_pool(name="psStat", bufs=1, space="PSUM") as psStat,
                    tc.tile_pool(name="lnP", bufs=2) as lnP,
                ):
                    t1 = pC.tile([P, DC, QT], f32)
                    t1bf = pC.tile([P, DC, QT], bf)
                    t1sq = pC.tile([P, DC, QT], bf)
                    # Wo projection + bo + residual
                    for mi in range(DC):
                        wo_t = wC.tile([P, DC, P], bf, name="wo_t", tag="w")
                        nc.sync.dma_start(out=wo_t, in_=wo_d[mi, :, :, :])
                        ps = psC.tile([P, QT], f32, name="psC", tag="ps")
                        for kc in range(DC):
                            nc.tensor.matmul(
                                ps,
                                lhsT=wo_t[:, kc, :],
                                rhs=attnT[:, kc, :],
                                start=(kc == 0),
                                stop=(kc == DC - 1),
                            )
                        nc.vector.scalar_tensor_tensor(
                            out=t1[:, mi, :],
                            in0=ps,
                            scalar=bo_t[:, mi : mi + 1],
                            in1=xqTf[:, mi, :],
                            op0=OP.add,
                            op1=OP.add,
                        )
                    _layernorm(
                        nc, tc, dramp, psStat, lnP, ones_bf, eps_t,
                        t1, t1bf, t1sq, g1_t, be1_t, u_f, u_bf, "ln1",
                    )

                with (
                    tc.tile_pool(name="poolD", bufs=1) as pD,
                    tc.tile_pool(name="wD", bufs=3) as wD,
                    tc.tile_pool(name="w2D", bufs=2) as w2D,
                    tc.tile_pool(name="psD", bufs=3, space="PSUM") as psD,
                    tc.tile_pool(name="psStat2", bufs=1, space="PSUM") as psStat2,
                    tc.tile_pool(name="lnP2", bufs=2) as lnP2,
                ):
                    hT = pD.tile([P, FC, QT], bf)
                    t2 = pD.tile([P, DC, QT], f32)
                    t2bf = pD.tile([P, DC, QT], bf)
                    t2sq = pD.tile([P, DC, QT], bf)
                    oT_sb = pD.tile([P, DC, QT], f32)
                    # FFN1: hT = gelu(W1^T u + b1)
                    for mi in range(FC):
                        w1_t = wD.tile([P, DC, P], bf, name="w1_t", tag="w")
                        nc.sync.dma_start(out=w1_t, in_=w1_d[mi, :, :, :])
                        ps = psD.tile([P, QT], f32, name="psD1", tag="ps")
                        for kc in range(DC):
                            nc.tensor.matmul(
                                ps,
                                lhsT=w1_t[:, kc, :],
                                rhs=u_bf[:, kc, :],
                                start=(kc == 0),
                                stop=(kc == DC - 1),
                            )
                        nc.scalar.activation(
                            hT[:, mi, :], ps, AF.Gelu, bias=b1_t[:, mi : mi + 1]
                        )
                    # FFN2 + b2 + residual u
                    for mi in range(DC):
                        w2_t = w2D.tile([P, FC, P], bf, name="w2_t", tag="w2")
                        nc.sync.dma_start(out=w2_t, in_=w2_d[mi, :, :, :])
                        ps = psD.tile([P, QT], f32, name="psD2", tag="ps")
                        for kc in range(FC):
                            nc.tensor.matmul(
                                ps,
                                lhsT=w2_t[:, kc, :],
                                rhs=hT[:, kc, :],
                                start=(kc == 0),
                                stop=(kc == FC - 1),
                            )
                        nc.vector.scalar_tensor_tensor(
                            out=t2[:, mi, :],
                            in0=ps,
                            scalar=b2_t[:, mi : mi + 1],
                            in1=u_f[:, mi, :],
                            op0=OP.add,
                            op1=OP.add,
                        )
                    _layernorm(
                        nc, tc, dramp, psStat2, lnP2, ones_bf, eps_t,
                        t2, t2bf, t2sq, g2_t, be2_t, oT_sb, None, "ln2",
                    )
                    for ci in range(DC):
                        nc.sync.dma_start(out=oT_d[:, ci, :], in_=oT_sb[:, ci, :])


def _layernorm(nc, tc, dramp, psStat, lnP, ones_bf, eps_t, t, tbf, tsq, g_t, be_t, out_f, out_bf, nm):
    """LN over d (partition+chunk axes) of transposed activation t [P, DC, QT].

    Writes out_f (f32) and optionally out_bf (bf16 copy).
    Stats via PE ones-matmuls on bf16 copies; mean/rstd broadcast via DRAM
    bounce; normalize + affine on DVE.
    """
    for mi in range(DC):
        nc.vector.tensor_copy(tbf[:, mi, :], t[:, mi, :])
        nc.vector.tensor_mul(tsq[:, mi, :], tbf[:, mi, :], tbf[:, mi, :])
    mu_ps = psStat.tile([1, QT], f32, name=f"mu_{nm}", tag="mu")
    sq_ps = psStat.tile([1, QT], f32, name=f"sq_{nm}", tag="sq")
    for mi in range(DC):
        nc.tensor.matmul(
            mu_ps, lhsT=ones_bf, rhs=tbf[:, mi, :],
            start=(mi == 0), stop=(mi == DC - 1),
        )
    for mi in range(DC):
        nc.tensor.matmul(
            sq_ps, lhsT=ones_bf, rhs=tsq[:, mi, :],
            start=(mi == 0), stop=(mi == DC - 1),
        )
    mean = lnP.tile([1, QT], f32, name=f"mean_{nm}", tag="r1")
    msq = lnP.tile([1, QT], f32, name=f"msq_{nm}", tag="r2")
    nc.scalar.mul(mean, mu_ps, 1.0 / D)
    nc.scalar.mul(msq, sq_ps, 1.0 / D)
    m2 = lnP.tile([1, QT], f32, name=f"m2_{nm}", tag="r3")
    nc.vector.tensor_mul(m2, mean, mean)
    var = lnP.tile([1, QT], f32, name=f"var_{nm}", tag="r4")
    nc.vector.tensor_sub(var, msq, m2)
    sd = lnP.tile([1, QT], f32, name=f"sd_{nm}", tag="r5")
    nc.scalar.activation(sd, var, AF.Sqrt, bias=eps_t[0:1, 0:1])
    rstd = lnP.tile([1, QT], f32, name=f"rstd_{nm}", tag="r6")
    nc.vector.reciprocal(rstd, sd)
    # broadcast mean and rstd to [P, QT] via DRAM bounce
    bnc_m = dramp.tile([1, QT], f32, name=f"bncm_{nm}", tag="bnc")
    bnc_r = dramp.tile([1, QT], f32, name=f"bncr_{nm}", tag="bnc")
    nc.sync.dma_start(out=bnc_m, in_=mean)
    nc.sync.dma_start(out=bnc_r, in_=rstd)
    mean_b = lnP.tile([P, QT], f32, name=f"meanb_{nm}", tag="b1")
    rstd_b = lnP.tile([P, QT], f32, name=f"rstdb_{nm}", tag="b2")
    nc.sync.dma_start(out=mean_b, in_=bnc_m[0:1, :].to_broadcast([P, QT]))
    nc.sync.dma_start(out=rstd_b, in_=bnc_r[0:1, :].to_broadcast([P, QT]))
    for mi in range(DC):
        cen = lnP.tile([P, QT], f32, name=f"cen_{nm}", tag="cen")
        nc.vector.tensor_sub(cen, t[:, mi, :], mean_b)
        nrm = lnP.tile([P, QT], f32, name=f"nrm_{nm}", tag="nrm")
        nc.vector.tensor_mul(nrm, cen, rstd_b)
        nc.vector.tensor_scalar(
            out=out_f[:, mi, :],
            in0=nrm,
            scalar1=g_t[:, mi : mi + 1],
            scalar2=be_t[:, mi : mi + 1],
            op0=OP.mult,
            op1=OP.add,
        )
        if out_bf is not None:
            nc.vector.tensor_copy(out_bf[:, mi, :], out_f[:, mi, :])


def _get_nc():
    global _NC_CACHE
    if _NC_CACHE is None:
        _NC_CACHE = _build_nc()
    return _NC_CACHE


def _prep_shared(inputs):
    bf16 = ml_dtypes.bfloat16

    def f(a):
        return np.ascontiguousarray(a, dtype=np.float32)

    Wq, Wk, Wv, Wo = f(inputs["Wq"]), f(inputs["Wk"]), f(inputs["Wv"]), f(inputs["Wo"])
    W1, W2 = f(inputs["W1"]), f(inputs["W2"])
    shared = {
        "wq": np.ascontiguousarray(
            Wq.reshape(DC, P, DC, P).transpose(2, 1, 0, 3)
        ).astype(bf16),
        "wk": np.ascontiguousarray(
            Wk.reshape(DC, P, DC, P).transpose(2, 1, 0, 3)
        ).astype(bf16),
        "wv": np.ascontiguousarray(Wv.reshape(DC, P, D).transpose(1, 0, 2)).astype(
            bf16
        ),
        "wo": np.ascontiguousarray(
            Wo.reshape(DC, P, DC, P).transpose(2, 1, 0, 3)
        ).astype(bf16),
        "w1": np.ascontiguousarray(
            W1.reshape(DC, P, FC, P).transpose(2, 1, 0, 3)
        ).astype(bf16),
        "w2": np.ascontiguousarray(
            W2.reshape(FC, P, DC, P).transpose(2, 1, 0, 3)
        ).astype(bf16),
        "bq": np.ascontiguousarray(f(inputs["bq"]).reshape(DC, P).T),
        "bk": np.ascontiguousarray(f(inputs["bk"]).reshape(DC, P).T),
        "bvb": np.ascontiguousarray(np.broadcast_to(f(inputs["bv"]), (P, D))),
        "bo": np.ascontiguousarray(f(inputs["bo"]).reshape(DC, P).T),
        "b1": np.ascontiguousarray(f(inputs["b1"]).reshape(FC, P).T),
        "b2": np.ascontiguousarray(f(inputs["b2"]).reshape(DC, P).T),
        "g1": np.ascontiguousarray(f(inputs["g1"]).reshape(DC, P).T),
        "be1": np.ascontiguousarray(f(inputs["beta1"]).reshape(DC, P).T),
        "g2": np.ascontiguousarray(f(inputs["g2"]).reshape(DC, P).T),
        "be2": np.ascontiguousarray(f(inputs["beta2"]).reshape(DC, P).T),
    }
    return shared


def kernel(**inputs):
    bf16 = ml_dtypes.bfloat16
    x = np.ascontiguousarray(inputs["x"], dtype=np.float32)  # [2, 2048, 1024]
    B = x.shape[0]
    n_cores = 8
    per_batch = n_cores // B  # 4

    nc = _get_nc()
    shared = _prep_shared(inputs)

    in_maps = []
    xT_cache = {}
    for c in range(n_cores):
        b = c // per_batch
        qs = (c % per_batch) * QT
        if b not in xT_cache:
            xb = x[b]  # [S, D]
            xT_cache[b] = np.ascontiguousarray(
                xb.T.reshape(DC, P, S).transpose(1, 0, 2)
            )
        xTf = xT_cache[b]
        xq = x[b][qs : qs + QT]  # [QT, D]
        xqTf = np.ascontiguousarray(xq.T.reshape(DC, P, QT).transpose(1, 0, 2))
        m = dict(shared)
        m["xT"] = xTf.astype(bf16)
        m["xqT"] = xqTf.astype(bf16)
        m["xqTf"] = xqTf
        in_maps.append(m)

    res = run_bass_kernel_spmd(nc, in_maps, core_ids=list(range(n_cores)))

    out = np.empty((B, S, D), dtype=np.float32)
    for c in range(n_cores):
        b = c // per_batch
        qs = (c % per_batch) * QT
        oT = res.results[c]["oT"]  # [P, DC, QT]
        out[b, qs : qs + QT, :] = oT.transpose(2, 1, 0).reshape(QT, D)
    return out


# revision 8
# speedup vs baseline: 24.2221x; 24.2221x over previous
"""Trainium2 Bass kernel for nn_Block_41893111005254 (dense transformer block).

Reference computation (per batch b of 2, seq 2048, d 1024, 16 heads, ff 4096):
    x = x + MHSA(x)           (no 1/sqrt(hd) scaling)
    x = LN(x, g1, beta1)
    x = x + gelu(x @ W1 + b1) @ W2 + b2
    x = LN(x, g2, beta2)

Sharding: sequence-parallel, collective-free. Core c handles batch c//4,
query tokens (c%4)*512 .. +512. Each core recomputes K/V for its batch's full
sequence (redundant across the 4 cores of a batch but avoids collectives).

All on-chip activations are kept TRANSPOSED ([d, token] with d partition-
chunked) so that:
  - all matmuls consume natural-layout bf16 weights as lhsT
  - LayerNorm reductions (over d) run on the TensorEngine via ones-matmuls
  - softmax denominators come free via a ones-column appended to V
  - no transposes are needed anywhere
"""

import sys

sys.path.insert(0, "/opt/trn_rl_repo")

import numpy as np
import ml_dtypes

import concourse.bacc as bacc
import concourse.tile as tile
from concourse import mybir
from concourse.bass_utils import run_bass_kernel_spmd

P = 128
D = 1024
S = 2048
QT = 512  # query tokens per core
FF = 4096
H = 16
HD = 64
DC = D // P  # 8 chunks of d
FC = FF // P  # 32 chunks of ff
NKC = S // P  # 16 kv token chunks
LN_EPS = 1e-5

bf = mybir.dt.bfloat16
f32 = mybir.dt.float32
AF = mybir.ActivationFunctionType
OP = mybir.AluOpType

_NC_CACHE = None


def _build_nc():
    nc = bacc.Bacc(None, target_bir_lowering=False, debug=False)

    # ---- I/O declarations (per-core shards, host-prepped layouts) ----
    xT_d = nc.dram_tensor("xT", [P, DC, S], bf, kind="ExternalInput")
    xqT_d = nc.dram_tensor("xqT", [P, DC, QT], bf, kind="ExternalInput")
    xqTf_d = nc.dram_tensor("xqTf", [P, DC, QT], f32, kind="ExternalInput")
    wq_d = nc.dram_tensor("wq", [DC, P, DC, P], bf, kind="ExternalInput")
    wk_d = nc.dram_tensor("wk", [DC, P, DC, P], bf, kind="ExternalInput")
    wv_d = nc.dram_tensor("wv", [P, DC, D], bf, kind="ExternalInput")
    wo_d = nc.dram_tensor("wo", [DC, P, DC, P], bf, kind="ExternalInput")
    w1_d = nc.dram_tensor("w1", [FC, P, DC, P], bf, kind="ExternalInput")
    w2_d = nc.dram_tensor("w2", [DC, P, FC, P], bf, kind="ExternalInput")
    bq_d = nc.dram_tensor("bq", [P, DC], f32, kind="ExternalInput")
    bk_d = nc.dram_tensor("bk", [P, DC], f32, kind="ExternalInput")
    bvb_d = nc.dram_tensor("bvb", [P, D], f32, kind="ExternalInput")
    bo_d = nc.dram_tensor("bo", [P, DC], f32, kind="ExternalInput")
    b1_d = nc.dram_tensor("b1", [P, FC], f32, kind="ExternalInput")
    b2_d = nc.dram_tensor("b2", [P, DC], f32, kind="ExternalInput")
    g1_d = nc.dram_tensor("g1", [P, DC], f32, kind="ExternalInput")
    be1_d = nc.dram_tensor("be1", [P, DC], f32, kind="ExternalInput")
    g2_d = nc.dram_tensor("g2", [P, DC], f32, kind="ExternalInput")
    be2_d = nc.dram_tensor("be2", [P, DC], f32, kind="ExternalInput")
    oT_d = nc.dram_tensor("oT", [P, DC, QT], f32, kind="ExternalOutput")

    with tile.TileContext(nc) as tc:
        _emit(nc, tc, locals())
    nc.compile()
    return nc


def _emit(nc, tc, d):
    xT_d, xqT_d, xqTf_d = d["xT_d"], d["xqT_d"], d["xqTf_d"]
    wq_d, wk_d, wv_d, wo_d, w1_d, w2_d = (
        d["wq_d"], d["wk_d"], d["wv_d"], d["wo_d"], d["w1_d"], d["w2_d"],
    )
    bq_d, bk_d, bvb_d, bo_d, b1_d, b2_d = (
        d["bq_d"], d["bk_d"], d["bvb_d"], d["bo_d"], d["b1_d"], d["b2_d"],
    )
    g1_d, be1_d, g2_d, be2_d, oT_d = (
        d["g1_d"], d["be1_d"], d["g2_d"], d["be2_d"], d["oT_d"],
    )

    from contextlib import ExitStack

    with ExitStack() as ctx:
        constp = ctx.enter_context(tc.tile_pool(name="const", bufs=1))
        dramp = ctx.enter_context(tc.tile_pool(name="drb", bufs=4, space="DRAM"))

        # ---- constants ----
        bq_t = constp.tile([P, DC], f32)
        bk_t = constp.tile([P, DC], f32)
        bvb_t = constp.tile([P, D], f32)
        bo_t = constp.tile([P, DC], f32)
        b1_t = constp.tile([P, FC], f32)
        b2_t = constp.tile([P, DC], f32)
        g1_t = constp.tile([P, DC], f32)
        be1_t = constp.tile([P, DC], f32)
        g2_t = constp.tile([P, DC], f32)
        be2_t = constp.tile([P, DC], f32)
        ones_bf = constp.tile([P, 1], bf)
        eps_t = constp.tile([1, 1], f32)
        nc.sync.dma_start(out=bq_t, in_=bq_d[:, :])
        nc.sync.dma_start(out=bk_t, in_=bk_d[:, :])
        nc.sync.dma_start(out=bvb_t, in_=bvb_d[:, :])
        nc.sync.dma_start(out=bo_t, in_=bo_d[:, :])
        nc.sync.dma_start(out=b1_t, in_=b1_d[:, :])
        nc.sync.dma_start(out=b2_t, in_=b2_d[:, :])
        nc.sync.dma_start(out=g1_t, in_=g1_d[:, :])
        nc.sync.dma_start(out=be1_t, in_=be1_d[:, :])
        nc.sync.dma_start(out=g2_t, in_=g2_d[:, :])
        nc.sync.dma_start(out=be2_t, in_=be2_d[:, :])
        nc.vector.memset(ones_bf, 1.0)
        nc.vector.memset(eps_t, LN_EPS)

        # ================= persistent across A..C =================
        with tc.tile_pool(name="poolAC", bufs=1) as pAC:
            xqTf = pAC.tile([P, DC, QT], f32)
            attnT = pAC.tile([P, DC, QT], bf)
            nc.sync.dma_start(out=xqTf, in_=xqTf_d[:, :, :])

            # ================= A..B: K^T, Q^T, V =================
            with tc.tile_pool(name="poolAB", bufs=1) as pAB:
                kT = pAB.tile([P, DC, S], bf)
                qT = pAB.tile([P, DC, QT], bf)
                v65 = pAB.tile([P, NKC, H, HD + 1], bf)

                with (
                    tc.tile_pool(name="poolA", bufs=1) as pA,
                    tc.tile_pool(name="wA", bufs=3) as wA,
                    tc.tile_pool(name="psA", bufs=3, space="PSUM") as psA,
                ):
                    xT = pA.tile([P, DC, S], bf)
                    xqT = pA.tile([P, DC, QT], bf)
                    wv_sb = pA.tile([P, DC, D], bf)
                    for ci in range(DC):
                        nc.sync.dma_start(out=xT[:, ci, :], in_=xT_d[:, ci, :])
                    for ci in range(0, DC, 2):
                        nc.sync.dma_start(
                            out=xqT[:, ci : ci + 2, :], in_=xqT_d[:, ci : ci + 2, :]
                        )
                        nc.sync.dma_start(
                            out=wv_sb[:, ci : ci + 2, :], in_=wv_d[:, ci : ci + 2, :]
                        )
                    # ones columns of V (data cols overwritten by evictions;
                    # whole-tile memset avoids 1-wide strided bf16 writes)
                    nc.vector.memset(v65, 1.0)

                    # ---- K^T = (x @ Wk)^T + bk, full batch seq ----
                    for mi in range(DC):
                        wk_t = wA.tile([P, DC, P], bf, name="wk_t", tag="w")
                        nc.sync.dma_start(out=wk_t, in_=wk_d[mi, :, :, :])
                        for th in range(2):  # token halves of 1024
                            ps = psA.tile([P, 1024], f32, name="psA", tag="ps")
                            for sub in range(2):
                                t0 = th * 1024 + sub * 512
                                for kc in range(DC):
                                    nc.tensor.matmul(
                                        ps[:, sub * 512 : (sub + 1) * 512],
                                        lhsT=wk_t[:, kc, :],
                                        rhs=xT[:, kc, t0 : t0 + 512],
                                        start=(kc == 0),
                                        stop=(kc == DC - 1),
                                    )
                            nc.scalar.activation(
                                kT[:, mi, th * 1024 : (th + 1) * 1024],
                                ps,
                                AF.Identity,
                                bias=bk_t[:, mi : mi + 1],
                            )

                    # ---- Q^T = (xq @ Wq)^T + bq ----
                    for mi in range(0, DC, 2):
                        wq_t = wA.tile([P, 2, DC, P], bf, name="wq_t", tag="w")
                        nc.sync.dma_start(
                            out=wq_t,
                            in_=wq_d[mi : mi + 2, :, :, :].rearrange(
                                "a p k m -> p a k m"
                            ),
                        )
                        ps = psA.tile([P, 1024], f32, name="psAq", tag="ps")
                        for sub in range(2):
                            for kc in range(DC):
                                nc.tensor.matmul(
                                    ps[:, sub * 512 : (sub + 1) * 512],
                                    lhsT=wq_t[:, sub, kc, :],
                                    rhs=xqT[:, kc, :],
                                    start=(kc == 0),
                                    stop=(kc == DC - 1),
                                )
                            nc.scalar.activation(
                                qT[:, mi + sub, :],
                                ps[:, sub * 512 : (sub + 1) * 512],
                                AF.Identity,
                                bias=bq_t[:, mi + sub : mi + sub + 1],
                            )

                    # ---- V natural + bv, ones col kept ----
                    for tc16 in range(NKC):
                        ps = psA.tile([P, 1024], f32, name="psAv", tag="ps")
                        for nh in range(2):
                            for kc in range(DC):
                                nc.tensor.matmul(
                                    ps[:, nh * 512 : (nh + 1) * 512],
                                    lhsT=xT[:, kc, tc16 * P : (tc16 + 1) * P],
                                    rhs=wv_sb[:, kc, nh * 512 : (nh + 1) * 512],
                                    start=(kc == 0),
                                    stop=(kc == DC - 1),
                                )
                        nc.vector.tensor_add(
                            v65[:, tc16, :, 0:HD],
                            ps.rearrange("p (h x) -> p h x", x=HD),
                            bvb_t.rearrange("p (h x) -> p h x", x=HD),
                        )

                # ================= B: attention =================
                with (
                    tc.tile_pool(name="poolB", bufs=3) as pB,
                    tc.tile_pool(name="expP", bufs=4) as expP,
                    tc.tile_pool(name="psS", bufs=2, space="PSUM") as psS,
                    tc.tile_pool(name="psO", bufs=4, space="PSUM") as psO,
                ):
                    for hp in range(H // 2):
                        h0, h1 = 2 * hp, 2 * hp + 1
                        o_ps = [
                            psO.tile([HD + 1, QT], f32, name=f"o{h}", tag="o")
                            for h in (h0, h1)
                        ]
                        for kc in range(NKC):
                            s01 = psS.tile([P, 1024], f32, name="s01", tag="s")
                            for i, h in enumerate((h0, h1)):
                                p0 = 64 * (h % 2)
                                nc.tensor.matmul(
                                    s01[:, i * 512 : (i + 1) * 512],
                                    lhsT=kT[p0 : p0 + 64, hp, kc * P : (kc + 1) * P],
                                    rhs=qT[p0 : p0 + 64, hp, :],
                                    start=True,
                                    stop=True,
                                )
                            e01 = expP.tile([P, 1024], bf, name="e01", tag="e")
                            nc.scalar.activation(e01, s01, AF.Exp)
                            for i, h in enumerate((h0, h1)):
                                nc.tensor.matmul(
                                    o_ps[i],
                                    lhsT=v65[:, kc, h, :],
                                    rhs=e01[:, i * 512 : (i + 1) * 512],
                                    start=(kc == 0),
                                    stop=(kc == NKC - 1),
                                )
                        # normalize: recip of den row, DRAM-bounce broadcast
                        for i, h in enumerate((h0, h1)):
                            rr = pB.tile([HD + 1, QT], f32, name="rr", tag="rr")
                            nc.vector.reciprocal(
                                rr[HD : HD + 1, :], o_ps[i][HD : HD + 1, :]
                            )
                            bnc = dramp.tile([1, QT], f32, name="bnc", tag="bnc")
                            nc.sync.dma_start(out=bnc, in_=rr[HD : HD + 1, :])
                            rb = pB.tile([HD, QT], f32, name="rb", tag="rb")
                            nc.sync.dma_start(
                                out=rb, in_=bnc[0:1, :].to_broadcast([HD, QT])
                            )
                            if h % 2 == 0:
                                nc.vector.tensor_mul(
                                    attnT[0:HD, hp, :], o_ps[i][0:HD, :], rb
                                )
                            else:
                                tmp = pB.tile([HD, QT], bf, name="tmpo", tag="tmpo")
                                nc.vector.tensor_mul(tmp, o_ps[i][0:HD, :], rb)
                                nc.sync.dma_start(out=attnT[HD:P, hp, :], in_=tmp)

            # ================= C..D: u = LN1(x + proj), FFN, LN2 =========
            with tc.tile_pool(name="poolCD", bufs=1) as pCD:
                u_f = pCD.tile([P, DC, QT], f32)
                u_bf = pCD.tile([P, DC, QT], bf)

                with (
                    tc.tile_pool(name="poolC", bufs=1) as pC,
                    tc.tile_pool(name="wC", bufs=3) as wC,
                    tc.tile_pool(name="psC", bufs=3, space="PSUM") as psC,
                    tc.tile_pool(name="psStat", bufs=1, space="PSUM") as psStat,
                    tc.tile_pool(name="lnP", bufs=2) as lnP,
                ):
                    t1 = pC.tile([P, DC, QT], f32)
                    t1bf = pC.tile([P, DC, QT], bf)
                    t1sq = pC.tile([P, DC, QT], bf)
                    # Wo projection + bo + residual
                    for mi in range(DC):
                        wo_t = wC.tile([P, DC, P], bf, name="wo_t", tag="w")
                        nc.sync.dma_start(out=wo_t, in_=wo_d[mi, :, :, :])
                        ps = psC.tile([P, QT], f32, name="psC", tag="ps")
                        for kc in range(DC):
                            nc.tensor.matmul(
                                ps,
                                lhsT=wo_t[:, kc, :],
                                rhs=attnT[:, kc, :],
                                start=(kc == 0),
                                stop=(kc == DC - 1),
                            )
                        nc.vector.scalar_tensor_tensor(
                            out=t1[:, mi, :],
                            in0=ps,
                            scalar=bo_t[:, mi : mi + 1],
                            in1=xqTf[:, mi, :],
                            op0=OP.add,
                            op1=OP.add,
                        )
                    _layernorm(
                        nc, tc, dramp, psStat, lnP, ones_bf, eps_t,
                        t1, t1bf, t1sq, g1_t, be1_t, u_f, u_bf, "ln1",
                    )

                with (
                    tc.tile_pool(name="poolD", bufs=1) as pD,
                    tc.tile_pool(name="wD", bufs=3) as wD,
                    tc.tile_pool(name="w2D", bufs=2) as w2D,
                    tc.tile_pool(name="psD", bufs=3, space="PSUM") as psD,
                    tc.tile_pool(name="psStat2", bufs=1, space="PSUM") as psStat2,
                    tc.tile_pool(name="lnP2", bufs=2) as lnP2,
                ):
                    hT = pD.tile([P, FC, QT], bf)
                    t2 = pD.tile([P, DC, QT], f32)
                    t2bf = pD.tile([P, DC, QT], bf)
                    t2sq = pD.tile([P, DC, QT], bf)
                    oT_sb = pD.tile([P, DC, QT], f32)
                    # FFN1: hT = gelu(W1^T u + b1)
                    for mi in range(FC):
                        w1_t = wD.tile([P, DC, P], bf, name="w1_t", tag="w")
                        nc.sync.dma_start(out=w1_t, in_=w1_d[mi, :, :, :])
                        ps = psD.tile([P, QT], f32, name="psD1", tag="ps")
                        for kc in range(DC):
                            nc.tensor.matmul(
                                ps,
                                lhsT=w1_t[:, kc, :],
                                rhs=u_bf[:, kc, :],
                                start=(kc == 0),
                                stop=(kc == DC - 1),
                            )
                        nc.scalar.activation(
                            hT[:, mi, :], ps, AF.Gelu, bias=b1_t[:, mi : mi + 1]
                        )
                    # FFN2 + b2 + residual u
                    for mi in range(DC):
                        w2_t = w2D.tile([P, FC, P], bf, name="w2_t", tag="w2")
                        nc.sync.dma_start(out=w2_t, in_=w2_d[mi, :, :, :])
                        ps = psD.tile([P, QT], f32, name="psD2", tag="ps")
                        for kc in range(FC):
                            nc.tensor.matmul(
                                ps,
                                lhsT=w2_t[:, kc, :],
                                rhs=hT[:, kc, :],
                                start=(kc == 0),
                                stop=(kc == FC - 1),
                            )
                        nc.vector.scalar_tensor_tensor(
                            out=t2[:, mi, :],
                            in0=ps,
                            scalar=b2_t[:, mi : mi + 1],
                            in1=u_f[:, mi, :],
                            op0=OP.add,
                            op1=OP.add,
                        )
                    _layernorm(
                        nc, tc, dramp, psStat2, lnP2, ones_bf, eps_t,
                        t2, t2bf, t2sq, g2_t, be2_t, oT_sb, None, "ln2",
                    )
                    for ci in range(DC):
                        nc.sync.dma_start(out=oT_d[:, ci, :], in_=oT_sb[:, ci, :])


def _layernorm(nc, tc, dramp, psStat, lnP, ones_bf, eps_t, t, tbf, tsq, g_t, be_t, out_f, out_bf, nm):
    """LN over d (partition+chunk axes) of transposed activation t [P, DC, QT].

    Writes out_f (f32) and optionally out_bf (bf16 copy).
    Stats via PE ones-matmuls on bf16 copies; mean/rstd broadcast via DRAM
    bounce; normalize + affine on DVE.
    """
    for mi in range(DC):
        nc.vector.tensor_copy(tbf[:, mi, :], t[:, mi, :])
        nc.vector.tensor_mul(tsq[:, mi, :], tbf[:, mi, :], tbf[:, mi, :])
    mu_ps = psStat.tile([1, QT], f32, name=f"mu_{nm}", tag="mu")
    sq_ps = psStat.tile([1, QT], f32, name=f"sq_{nm}", tag="sq")
    for mi in range(DC):
        nc.tensor.matmul(
            mu_ps, lhsT=ones_bf, rhs=tbf[:, mi, :],
            start=(mi == 0), stop=(mi == DC - 1),
        )
    for mi in range(DC):
        nc.tensor.matmul(
            sq_ps, lhsT=ones_bf, rhs=tsq[:, mi, :],
            start=(mi == 0), stop=(mi == DC - 1),
        )
    mean = lnP.tile([1, QT], f32, name=f"mean_{nm}", tag="r1")
    msq = lnP.tile([1, QT], f32, name=f"msq_{nm}", tag="r2")
    nc.scalar.mul(mean, mu_ps, 1.0 / D)
    nc.scalar.mul(msq, sq_ps, 1.0 / D)
    m2 = lnP.tile([1, QT], f32, name=f"m2_{nm}", tag="r3")
    nc.vector.tensor_mul(m2, mean, mean)
    var = lnP.tile([1, QT], f32, name=f"var_{nm}", tag="r4")
    nc.vector.tensor_sub(var, msq, m2)
    sd = lnP.tile([1, QT], f32, name=f"sd_{nm}", tag="r5")
    nc.scalar.activation(sd, var, AF.Sqrt, bias=eps_t[0:1, 0:1])
    rstd = lnP.tile([1, QT], f32, name=f"rstd_{nm}", tag="r6")
    nc.vector.reciprocal(rstd, sd)
    # broadcast mean and rstd to [P, QT] via DRAM bounce
    bnc_m = dramp.tile([1, QT], f32, name=f"bncm_{nm}", tag="bnc")
    bnc_r = dramp.tile([1, QT], f32, name=f"bncr_{nm}", tag="bnc")
    nc.sync.dma_start(out=bnc_m, in_=mean)
    nc.sync.dma_start(out=bnc_r, in_=rstd)
    mean_b = lnP.tile([P, QT], f32, name=f"meanb_{nm}", tag="b1")
    rstd_b = lnP.tile([P, QT], f32, name=f"rstdb_{nm}", tag="b2")
    nc.sync.dma_start(out=mean_b, in_=bnc_m[0:1, :].to_broadcast([P, QT]))
    nc.sync.dma_start(out=rstd_b, in_=bnc_r[0:1, :].to_broadcast([P, QT]))
    for mi in range(DC):
        cen = lnP.tile([P, QT], f32, name=f"cen_{nm}", tag="cen")
        nc.vector.tensor_sub(cen, t[:, mi, :], mean_b)
        nrm = lnP.tile([P, QT], f32, name=f"nrm_{nm}", tag="nrm")
        nc.vector.tensor_mul(nrm, cen, rstd_b)
        nc.vector.tensor_scalar(
            out=out_f[:, mi, :],
            in0=nrm,
            scalar1=g_t[:, mi : mi + 1],
            scalar2=be_t[:, mi : mi + 1],
            op0=OP.mult,
            op1=OP.add,
        )
        if out_bf is not None:
            nc.vector.tensor_copy(out_bf[:, mi, :], out_f[:, mi, :])


def _get_nc():
    global _NC_CACHE
    if _NC_CACHE is None:
        _NC_CACHE = _build_nc()
    return _NC_CACHE


_RUNNER_CACHE = None


def _get_runner():
    """Cached jitted 8-core SPMD executor (avoids re-jitting per call).

    Modeled on bass2jax.run_bass_via_pjrt, but the jitted function and the
    sharding mesh are built once and reused.
    """
    global _RUNNER_CACHE
    if _RUNNER_CACHE is not None:
        return _RUNNER_CACHE

    import jax
    from jax.sharding import Mesh, PartitionSpec
    from jax.experimental.shard_map import shard_map
    from concourse import mybir as _mb
    from concourse.bass2jax import (
        _bass_exec_p,
        install_neuronx_cc_hook,
        partition_id_tensor,
    )

    install_neuronx_cc_hook()
    nc = _get_nc()
    n_cores = 8

    partition_name = nc.partition_id_tensor.name if nc.partition_id_tensor else None
    in_names = []
    out_names = []
    out_avals = []
    zero_outs = []
    for alloc in nc.m.functions[0].allocations:
        if not isinstance(alloc, _mb.MemoryLocationSet):
            continue
        name = alloc.memorylocations[0].name
        if alloc.kind == "ExternalInput":
            if name != partition_name:
                in_names.append(name)
        elif alloc.kind == "ExternalOutput":
            out_names.append(name)
            shape = tuple(alloc.tensor_shape)
            dtype = _mb.dt.np(alloc.dtype)
            out_avals.append(jax.core.ShapedArray(shape, dtype))
            zero_outs.append(np.zeros(shape, dtype))
    n_params = len(in_names)
    n_outs = len(out_avals)
    all_in_names = list(in_names) + list(out_names)
    if partition_name is not None:
        all_in_names.append(partition_name)

    def _body(*args):
        operands = list(args)
        if partition_name is not None:
            operands.append(partition_id_tensor())
        outs = _bass_exec_p.bind(
            *operands,
            out_avals=tuple(out_avals),
            in_names=tuple(all_in_names),
            out_names=tuple(out_names),
            lowering_input_output_aliases=(),
            sim_require_finite=True,
            sim_require_nnan=True,
            nc=nc,
        )
        return tuple(outs)

    devices = jax.devices()[:n_cores]
    mesh = Mesh(np.asarray(devices), ("core",))
    in_specs = (PartitionSpec("core"),) * (n_params + n_outs)
    out_specs = (PartitionSpec("core"),) * n_outs
    donate = tuple(range(n_params, n_params + n_outs))
    sharded = jax.jit(
        shard_map(
            _body, mesh=mesh, in_specs=in_specs, out_specs=out_specs, check_rep=False
        ),
        donate_argnums=donate,
        keep_unused=True,
    )

    _RUNNER_CACHE = {
        "fn": sharded,
        "in_names": in_names,
        "out_names": out_names,
        "out_avals": out_avals,
        "zero_outs": zero_outs,
        "n_cores": n_cores,
    }
    return _RUNNER_CACHE


def _run_spmd(in_maps):
    """Execute the kernel on 8 cores; returns list of per-core output dicts."""
    r = _get_runner()
    n_cores = r["n_cores"]
    concat_in = [
        np.concatenate([np.asarray(in_maps[c][name]) for c in range(n_cores)], axis=0)
        for name in r["in_names"]
    ]
    concat_zeros = [
        np.zeros((n_cores * z.shape[0], *z.shape[1:]), z.dtype) for z in r["zero_outs"]
    ]
    out_arrs = r["fn"](*concat_in, *concat_zeros)
    results = []
    for c in range(n_cores):
        results.append(
            {
                name: np.asarray(out_arrs[i]).reshape(
                    n_cores, *r["out_avals"][i].shape
                )[c]
                for i, name in enumerate(r["out_names"])
            }
        )
    return results


def _prep_shared(inputs):
    bf16 = ml_dtypes.bfloat16

    def f(a):
        return np.ascontiguousarray(a, dtype=np.float32)

    Wq, Wk, Wv, Wo = f(inputs["Wq"]), f(inputs["Wk"]), f(inputs["Wv"]), f(inputs["Wo"])
    W1, W2 = f(inputs["W1"]), f(inputs["W2"])
    shared = {
        "wq": np.ascontiguousarray(
            Wq.reshape(DC, P, DC, P).transpose(2, 1, 0, 3)
        ).astype(bf16),
        "wk": np.ascontiguousarray(
            Wk.reshape(DC, P, DC, P).transpose(2, 1, 0, 3)
        ).astype(bf16),
        "wv": np.ascontiguousarray(Wv.reshape(DC, P, D).transpose(1, 0, 2)).astype(
            bf16
        ),
        "wo": np.ascontiguousarray(
            Wo.reshape(DC, P, DC, P).transpose(2, 1, 0, 3)
        ).astype(bf16),
        "w1": np.ascontiguousarray(
            W1.reshape(DC, P, FC, P).transpose(2, 1, 0, 3)
        ).astype(bf16),
        "w2": np.ascontiguousarray(
            W2.reshape(FC, P, DC, P).transpose(2, 1, 0, 3)
        ).astype(bf16),
        "bq": np.ascontiguousarray(f(inputs["bq"]).reshape(DC, P).T),
        "bk": np.ascontiguousarray(f(inputs["bk"]).reshape(DC, P).T),
        "bvb": np.ascontiguousarray(np.broadcast_to(f(inputs["bv"]), (P, D))),
        "bo": np.ascontiguousarray(f(inputs["bo"]).reshape(DC, P).T),
        "b1": np.ascontiguousarray(f(inputs["b1"]).reshape(FC, P).T),
        "b2": np.ascontiguousarray(f(inputs["b2"]).reshape(DC, P).T),
        "g1": np.ascontiguousarray(f(inputs["g1"]).reshape(DC, P).T),
        "be1": np.ascontiguousarray(f(inputs["beta1"]).reshape(DC, P).T),
        "g2": np.ascontiguousarray(f(inputs["g2"]).reshape(DC, P).T),
        "be2": np.ascontiguousarray(f(inputs["beta2"]).reshape(DC, P).T),
    }
    return shared


def kernel(**inputs):
    bf16 = ml_dtypes.bfloat16
    x = np.ascontiguousarray(inputs["x"], dtype=np.float32)  # [2, 2048, 1024]
    B = x.shape[0]
    n_cores = 8
    per_batch = n_cores // B  # 4

    nc = _get_nc()
    shared = _prep_shared(inputs)

    in_maps = []
    xT_cache = {}
    for c in range(n_cores):
        b = c // per_batch
        qs = (c % per_batch) * QT
        if b not in xT_cache:
            xb = x[b]  # [S, D]
            xT_cache[b] = np.ascontiguousarray(
                xb.T.reshape(DC, P, S).transpose(1, 0, 2)
            )
        xTf = xT_cache[b]
        xq = x[b][qs : qs + QT]  # [QT, D]
        xqTf = np.ascontiguousarray(xq.T.reshape(DC, P, QT).transpose(1, 0, 2))
        m = dict(shared)
        m["xT"] = xTf.astype(bf16)
        m["xqT"] = xqTf.astype(bf16)
        m["xqTf"] = xqTf
        in_maps.append(m)

    results = _run_spmd(in_maps)

    out = np.empty((B, S, D), dtype=np.float32)
    for c in range(n_cores):
        b = c // per_batch
        qs = (c % per_batch) * QT
        oT = results[c]["oT"]  # [P, DC, QT]
        out[b, qs : qs + QT, :] = oT.transpose(2, 1, 0).reshape(QT, D)
    return out


# revision 23
# speedup vs baseline: 49094.0385x; 2026.8317x over previous
"""Trainium2 Bass kernel for nn_Block_41893111005254 (dense transformer block).

Reference computation (per batch b of 2, seq 2048, d 1024, 16 heads, ff 4096):
    x = x + MHSA(x)           (no 1/sqrt(hd) scaling)
    x = LN(x, g1, beta1)
    x = x + gelu(x @ W1 + b1) @ W2 + b2
    x = LN(x, g2, beta2)

Sharding: sequence-parallel, collective-free. Core c handles batch c//4,
query tokens (c%4)*512 .. +512. Each core recomputes K/V for its batch's full
sequence (redundant across the 4 cores of a batch but avoids collectives).

All on-chip activations are kept TRANSPOSED ([d, token] with d partition-
chunked) so that:
  - all matmuls consume natural-layout bf16 weights as lhsT
  - LayerNorm reductions (over d) run on the TensorEngine via ones-matmuls
  - softmax denominators come free via a ones-column appended to V
  - no transposes are needed anywhere
"""

import sys

sys.path.insert(0, "/opt/trn_rl_repo")

import numpy as np
import ml_dtypes

import concourse.bacc as bacc
import concourse.tile as tile
from concourse import mybir
from concourse.bass_utils import run_bass_kernel_spmd

P = 128
D = 1024
S = 2048
QT = 512  # query tokens per core
FF = 4096
H = 16
HD = 64
DC = D // P  # 8 chunks of d
FC = FF // P  # 32 chunks of ff
NKC = S // P  # 16 kv token chunks
LN_EPS = 1e-5

bf = mybir.dt.bfloat16
f32 = mybir.dt.float32
AF = mybir.ActivationFunctionType
OP = mybir.AluOpType

_NC_CACHE = None


def _build_nc(reps=1):
    nc = bacc.Bacc(None, target_bir_lowering=False, debug=False)

    # ---- I/O declarations (per-core shards, host-prepped layouts) ----
    xT_d = nc.dram_tensor("xT", [P, DC, S], bf, kind="ExternalInput")
    xqT_d = nc.dram_tensor("xqT", [P, DC, QT], bf, kind="ExternalInput")
    xqTf_d = nc.dram_tensor("xqTf", [P, DC, QT], f32, kind="ExternalInput")
    wq_d = nc.dram_tensor("wq", [DC, P, DC, P], bf, kind="ExternalInput")
    wk_d = nc.dram_tensor("wk", [DC, P, DC, P], bf, kind="ExternalInput")
    wv_d = nc.dram_tensor("wv", [P, DC, D], bf, kind="ExternalInput")
    wo_d = nc.dram_tensor("wo", [DC, P, DC, P], bf, kind="ExternalInput")
    w1_d = nc.dram_tensor("w1", [FC, P, DC, P], bf, kind="ExternalInput")
    w2_d = nc.dram_tensor("w2", [DC, P, FC, P], bf, kind="ExternalInput")
    bq_d = nc.dram_tensor("bq", [P, DC], f32, kind="ExternalInput")
    bk_d = nc.dram_tensor("bk", [P, DC], f32, kind="ExternalInput")
    bvb_d = nc.dram_tensor("bvb", [P, D], f32, kind="ExternalInput")
    bo_d = nc.dram_tensor("bo", [P, DC], f32, kind="ExternalInput")
    b1_d = nc.dram_tensor("b1", [P, FC], f32, kind="ExternalInput")
    b2_d = nc.dram_tensor("b2", [P, DC], f32, kind="ExternalInput")
    g1_d = nc.dram_tensor("g1", [P, DC], f32, kind="ExternalInput")
    be1_d = nc.dram_tensor("be1", [P, DC], f32, kind="ExternalInput")
    g2_d = nc.dram_tensor("g2", [P, DC], f32, kind="ExternalInput")
    be2_d = nc.dram_tensor("be2", [P, DC], f32, kind="ExternalInput")
    oT_d = nc.dram_tensor("oT", [P, DC, QT], f32, kind="ExternalOutput")

    tensors = dict(locals())
    with tile.TileContext(nc) as tc:
        for _rep in range(reps):
            _emit(nc, tc, tensors)
    nc.compile()
    return nc


def _emit(nc, tc, d):
    xT_d, xqT_d, xqTf_d = d["xT_d"], d["xqT_d"], d["xqTf_d"]
    wq_d, wk_d, wv_d, wo_d, w1_d, w2_d = (
        d["wq_d"], d["wk_d"], d["wv_d"], d["wo_d"], d["w1_d"], d["w2_d"],
    )
    bq_d, bk_d, bvb_d, bo_d, b1_d, b2_d = (
        d["bq_d"], d["bk_d"], d["bvb_d"], d["bo_d"], d["b1_d"], d["b2_d"],
    )
    g1_d, be1_d, g2_d, be2_d, oT_d = (
        d["g1_d"], d["be1_d"], d["g2_d"], d["be2_d"], d["oT_d"],
    )

    from contextlib import ExitStack

    with ExitStack() as ctx:
        constp = ctx.enter_context(tc.tile_pool(name="const", bufs=1))
        dramp = ctx.enter_context(tc.tile_pool(name="drb", bufs=4, space="DRAM"))

        # ---- constants ----
        bq_t = constp.tile([P, DC], f32)
        bk_t = constp.tile([P, DC], f32)
        bvb_t = constp.tile([P, D], f32)
        bo_t = constp.tile([P, DC], f32)
        b1_t = constp.tile([P, FC], f32)
        b2_t = constp.tile([P, DC], f32)
        g1_t = constp.tile([P, DC], f32)
        be1_t = constp.tile([P, DC], f32)
        g2_t = constp.tile([P, DC], f32)
        be2_t = constp.tile([P, DC], f32)
        ones_bf = constp.tile([P, 1], bf)
        eps_t = constp.tile([1, 1], f32)
        nc.sync.dma_start(out=bq_t, in_=bq_d[:, :])
        nc.sync.dma_start(out=bk_t, in_=bk_d[:, :])
        nc.sync.dma_start(out=bvb_t, in_=bvb_d[:, :])
        nc.sync.dma_start(out=bo_t, in_=bo_d[:, :])
        nc.sync.dma_start(out=b1_t, in_=b1_d[:, :])
        nc.sync.dma_start(out=b2_t, in_=b2_d[:, :])
        nc.sync.dma_start(out=g1_t, in_=g1_d[:, :])
        nc.sync.dma_start(out=be1_t, in_=be1_d[:, :])
        nc.sync.dma_start(out=g2_t, in_=g2_d[:, :])
        nc.sync.dma_start(out=be2_t, in_=be2_d[:, :])
        nc.vector.memset(ones_bf, 1.0)
        nc.vector.memset(eps_t, LN_EPS)

        # ================= persistent across A..C =================
        with tc.tile_pool(name="poolAC", bufs=1) as pAC:
            xqTf = pAC.tile([P, DC, QT], f32)
            attnT = pAC.tile([P, DC, QT], bf)
            nc.sync.dma_start(out=xqTf, in_=xqTf_d[:, :, :])

            # ================= A..B: K^T, Q^T, V =================
            with tc.tile_pool(name="poolAB", bufs=1) as pAB:
                kT = pAB.tile([P, DC, S], bf)
                qT = pAB.tile([P, DC, QT], bf)
                v65 = pAB.tile([P, NKC, H, HD + 1], bf)

                with (
                    tc.tile_pool(name="poolA", bufs=1) as pA,
                    tc.tile_pool(name="wA", bufs=3) as wA,
                    tc.tile_pool(name="psA", bufs=3, space="PSUM") as psA,
                ):
                    xT = pA.tile([P, DC, S], bf)
                    xqT = pA.tile([P, DC, QT], bf)
                    wv_sb = pA.tile([P, DC, D], bf)
                    for ci in range(DC):
                        nc.sync.dma_start(out=xT[:, ci, :], in_=xT_d[:, ci, :])
                    for ci in range(0, DC, 2):
                        nc.sync.dma_start(
                            out=xqT[:, ci : ci + 2, :], in_=xqT_d[:, ci : ci + 2, :]
                        )
                        nc.sync.dma_start(
                            out=wv_sb[:, ci : ci + 2, :], in_=wv_d[:, ci : ci + 2, :]
                        )
                    # ones columns of V (data cols overwritten by evictions;
                    # whole-tile memset avoids 1-wide strided bf16 writes)
                    nc.vector.memset(v65, 1.0)

                    # ---- K^T = (x @ Wk)^T + bk, full batch seq ----
                    for mi in range(DC):
                        wk_t = wA.tile([P, DC, P], bf, name="wk_t", tag="w")
                        nc.sync.dma_start(out=wk_t, in_=wk_d[mi, :, :, :])
                        for th in range(2):  # token halves of 1024
                            ps = psA.tile([P, 1024], f32, name="psA", tag="ps")
                            for sub in range(2):
                                t0 = th * 1024 + sub * 512
                                for kc in range(DC):
                                    nc.tensor.matmul(
                                        ps[:, sub * 512 : (sub + 1) * 512],
                                        lhsT=wk_t[:, kc, :],
                                        rhs=xT[:, kc, t0 : t0 + 512],
                                        start=(kc == 0),
                                        stop=(kc == DC - 1),
                                    )
                            nc.vector.tensor_scalar_add(
                                out=kT[:, mi, th * 1024 : (th + 1) * 1024],
                                in0=ps,
                                scalar1=bk_t[:, mi : mi + 1],
                            )

                    # ---- Q^T = (xq @ Wq)^T + bq ----
                    for mi in range(0, DC, 2):
                        wq_t = wA.tile([P, 2, DC, P], bf, name="wq_t", tag="w")
                        nc.sync.dma_start(
                            out=wq_t,
                            in_=wq_d[mi : mi + 2, :, :, :].rearrange(
                                "a p k m -> p a k m"
                            ),
                        )
                        ps = psA.tile([P, 1024], f32, name="psAq", tag="ps")
                        for sub in range(2):
                            for kc in range(DC):
                                nc.tensor.matmul(
                                    ps[:, sub * 512 : (sub + 1) * 512],
                                    lhsT=wq_t[:, sub, kc, :],
                                    rhs=xqT[:, kc, :],
                                    start=(kc == 0),
                                    stop=(kc == DC - 1),
                                )
                            nc.vector.tensor_scalar_add(
                                out=qT[:, mi + sub, :],
                                in0=ps[:, sub * 512 : (sub + 1) * 512],
                                scalar1=bq_t[:, mi + sub : mi + sub + 1],
                            )

                    # ---- V natural + bv, ones col kept ----
                    for tc16 in range(NKC):
                        ps = psA.tile([P, 1024], f32, name="psAv", tag="ps")
                        for nh in range(2):
                            for kc in range(DC):
                                nc.tensor.matmul(
                                    ps[:, nh * 512 : (nh + 1) * 512],
                                    lhsT=xT[:, kc, tc16 * P : (tc16 + 1) * P],
                                    rhs=wv_sb[:, kc, nh * 512 : (nh + 1) * 512],
                                    start=(kc == 0),
                                    stop=(kc == DC - 1),
                                )
                        nc.vector.tensor_add(
                            v65[:, tc16, :, 0:HD],
                            ps.rearrange("p (h x) -> p h x", x=HD),
                            bvb_t.rearrange("p (h x) -> p h x", x=HD),
                        )

                # ================= B: attention =================
                with (
                    tc.tile_pool(name="poolB", bufs=3) as pB,
                    tc.tile_pool(name="expP", bufs=4) as expP,
                    tc.tile_pool(name="psS", bufs=2, space="PSUM") as psS,
                    tc.tile_pool(name="psO", bufs=4, space="PSUM") as psO,
                ):
                    for hp in range(H // 2):
                        h0, h1 = 2 * hp, 2 * hp + 1
                        o_ps = [
                            psO.tile([HD + 1, QT], f32, name=f"o{h}", tag="o")
                            for h in (h0, h1)
                        ]
                        for kc in range(NKC):
                            s01 = psS.tile([P, 1024], f32, name="s01", tag="s")
                            for i, h in enumerate((h0, h1)):
                                p0 = 64 * (h % 2)
                                nc.tensor.matmul(
                                    s01[:, i * 512 : (i + 1) * 512],
                                    lhsT=kT[p0 : p0 + 64, hp, kc * P : (kc + 1) * P],
                                    rhs=qT[p0 : p0 + 64, hp, :],
                                    start=True,
                                    stop=True,
                                )
                            e01 = expP.tile([P, 1024], bf, name="e01", tag="e")
                            nc.scalar.activation(e01, s01, AF.Exp)
                            for i, h in enumerate((h0, h1)):
                                nc.tensor.matmul(
                                    o_ps[i],
                                    lhsT=v65[:, kc, h, :],
                                    rhs=e01[:, i * 512 : (i + 1) * 512],
                                    start=(kc == 0),
                                    stop=(kc == NKC - 1),
                                )
                        # normalize: recip of den row, DRAM-bounce broadcast
                        for i, h in enumerate((h0, h1)):
                            rr = pB.tile([HD + 1, QT], f32, name="rr", tag="rr")
                            nc.vector.reciprocal(
                                rr[HD : HD + 1, :], o_ps[i][HD : HD + 1, :]
                            )
                            bnc = dramp.tile([1, QT], f32, name="bnc", tag="bnc")
                            nc.sync.dma_start(out=bnc, in_=rr[HD : HD + 1, :])
                            rb = pB.tile([HD, QT], f32, name="rb", tag="rb")
                            nc.sync.dma_start(
                                out=rb, in_=bnc[0:1, :].to_broadcast([HD, QT])
                            )
                            if h % 2 == 0:
                                nc.vector.tensor_mul(
                                    attnT[0:HD, hp, :], o_ps[i][0:HD, :], rb
                                )
                            else:
                                tmp = pB.tile([HD, QT], bf, name="tmpo", tag="tmpo")
                                nc.vector.tensor_mul(tmp, o_ps[i][0:HD, :], rb)
                                nc.sync.dma_start(out=attnT[HD:P, hp, :], in_=tmp)

            # ================= C..D: u = LN1(x + proj), FFN, LN2 =========
            with tc.tile_pool(name="poolCD", bufs=1) as pCD:
                u_f = pCD.tile([P, DC, QT], f32)
                u_bf = pCD.tile([P, DC, QT], bf)

                with (
                    tc.tile_pool(name="poolC", bufs=1) as pC,
                    tc.tile_pool(name="wC", bufs=3) as wC,
                    tc.tile_pool(name="psC", bufs=3, space="PSUM") as psC,
                    tc.tile_pool(name="psStat", bufs=1, space="PSUM") as psStat,
                    tc.tile_pool(name="lnP", bufs=2) as lnP,
                ):
                    t1 = pC.tile([P, DC, QT], f32)
                    t1bf = pC.tile([P, DC, QT], bf)
                    t1sq = pC.tile([P, DC, QT], bf)
                    # Wo projection + bo + residual
                    for mi in range(DC):
                        wo_t = wC.tile([P, DC, P], bf, name="wo_t", tag="w")
                        nc.sync.dma_start(out=wo_t, in_=wo_d[mi, :, :, :])
                        ps = psC.tile([P, QT], f32, name="psC", tag="ps")
                        for kc in range(DC):
                            nc.tensor.matmul(
                                ps,
                                lhsT=wo_t[:, kc, :],
                                rhs=attnT[:, kc, :],
                                start=(kc == 0),
                                stop=(kc == DC - 1),
                            )
                        nc.vector.scalar_tensor_tensor(
                            out=t1[:, mi, :],
                            in0=ps,
                            scalar=bo_t[:, mi : mi + 1],
                            in1=xqTf[:, mi, :],
                            op0=OP.add,
                            op1=OP.add,
                        )
                    _layernorm(
                        nc, tc, dramp, psStat, lnP, ones_bf, eps_t,
                        t1, t1bf, t1sq, g1_t, be1_t, u_f, u_bf, "ln1",
                    )

                with (
                    tc.tile_pool(name="poolD", bufs=1) as pD,
                    tc.tile_pool(name="wD", bufs=3) as wD,
                    tc.tile_pool(name="w2D", bufs=2) as w2D,
                    tc.tile_pool(name="psD", bufs=3, space="PSUM") as psD,
                    tc.tile_pool(name="psStat2", bufs=1, space="PSUM") as psStat2,
                    tc.tile_pool(name="lnP2", bufs=2) as lnP2,
                ):
                    hT = pD.tile([P, FC, QT], bf)
                    t2 = pD.tile([P, DC, QT], f32)
                    t2bf = pD.tile([P, DC, QT], bf)
                    t2sq = pD.tile([P, DC, QT], bf)
                    oT_sb = pD.tile([P, DC, QT], f32)
                    # FFN1: hT = gelu(W1^T u + b1)
                    for mi in range(FC):
                        w1_t = wD.tile([P, DC, P], bf, name="w1_t", tag="w")
                        nc.sync.dma_start(out=w1_t, in_=w1_d[mi, :, :, :])
                        ps = psD.tile([P, QT], f32, name="psD1", tag="ps")
                        for kc in range(DC):
                            nc.tensor.matmul(
                                ps,
                                lhsT=w1_t[:, kc, :],
                                rhs=u_bf[:, kc, :],
                                start=(kc == 0),
                                stop=(kc == DC - 1),
                            )
                        nc.scalar.activation(
                            hT[:, mi, :], ps, AF.Gelu, bias=b1_t[:, mi : mi + 1]
                        )
                    # FFN2 + b2 + residual u
                    for mi in range(DC):
                        w2_t = w2D.tile([P, FC, P], bf, name="w2_t", tag="w2")
                        nc.sync.dma_start(out=w2_t, in_=w2_d[mi, :, :, :])
                        ps = psD.tile([P, QT], f32, name="psD2", tag="ps")
                        for kc in range(FC):
                            nc.tensor.matmul(
                                ps,
                                lhsT=w2_t[:, kc, :],
                                rhs=hT[:, kc, :],
                                start=(kc == 0),
                                stop=(kc == FC - 1),
                            )
                        nc.vector.scalar_tensor_tensor(
                            out=t2[:, mi, :],
                            in0=ps,
                            scalar=b2_t[:, mi : mi + 1],
                            in1=u_f[:, mi, :],
                            op0=OP.add,
                            op1=OP.add,
                        )
                    _layernorm(
                        nc, tc, dramp, psStat2, lnP2, ones_bf, eps_t,
                        t2, t2bf, t2sq, g2_t, be2_t, oT_sb, None, "ln2",
                    )
                    for ci in range(DC):
                        nc.sync.dma_start(out=oT_d[:, ci, :], in_=oT_sb[:, ci, :])


def _layernorm(nc, tc, dramp, psStat, lnP, ones_bf, eps_t, t, tbf, tsq, g_t, be_t, out_f, out_bf, nm):
    """LN over d (partition+chunk axes) of transposed activation t [P, DC, QT].

    Writes out_f (f32) and optionally out_bf (bf16 copy).
    Stats via PE ones-matmuls on bf16 copies; mean/rstd broadcast via DRAM
    bounce; normalize + affine on DVE.
    """
    for mi in range(DC):
        nc.vector.tensor_copy(tbf[:, mi, :], t[:, mi, :])
        nc.vector.tensor_mul(tsq[:, mi, :], tbf[:, mi, :], tbf[:, mi, :])
    mu_ps = psStat.tile([1, QT], f32, name=f"mu_{nm}", tag="mu")
    sq_ps = psStat.tile([1, QT], f32, name=f"sq_{nm}", tag="sq")
    for mi in range(DC):
        nc.tensor.matmul(
            mu_ps, lhsT=ones_bf, rhs=tbf[:, mi, :],
            start=(mi == 0), stop=(mi == DC - 1),
        )
    for mi in range(DC):
        nc.tensor.matmul(
            sq_ps, lhsT=ones_bf, rhs=tsq[:, mi, :],
            start=(mi == 0), stop=(mi == DC - 1),
        )
    mean = lnP.tile([1, QT], f32, name=f"mean_{nm}", tag="r1")
    msq = lnP.tile([1, QT], f32, name=f"msq_{nm}", tag="r2")
    nc.scalar.mul(mean, mu_ps, 1.0 / D)
    nc.scalar.mul(msq, sq_ps, 1.0 / D)
    m2 = lnP.tile([1, QT], f32, name=f"m2_{nm}", tag="r3")
    nc.vector.tensor_mul(m2, mean, mean)
    var = lnP.tile([1, QT], f32, name=f"var_{nm}", tag="r4")
    nc.vector.tensor_sub(var, msq, m2)
    sd = lnP.tile([1, QT], f32, name=f"sd_{nm}", tag="r5")
    nc.scalar.activation(sd, var, AF.Sqrt, bias=eps_t[0:1, 0:1])
    rstd = lnP.tile([1, QT], f32, name=f"rstd_{nm}", tag="r6")
    nc.vector.reciprocal(rstd, sd)
    # broadcast mean and rstd to [P, QT] via DRAM bounce
    bnc_m = dramp.tile([1, QT], f32, name=f"bncm_{nm}", tag="bnc")
    bnc_r = dramp.tile([1, QT], f32, name=f"bncr_{nm}", tag="bnc")
    nc.sync.dma_start(out=bnc_m, in_=mean)
    nc.sync.dma_start(out=bnc_r, in_=rstd)
    mean_b = lnP.tile([P, QT], f32, name=f"meanb_{nm}", tag="b1")
    rstd_b = lnP.tile([P, QT], f32, name=f"rstdb_{nm}", tag="b2")
    nc.sync.dma_start(out=mean_b, in_=bnc_m[0:1, :].to_broadcast([P, QT]))
    nc.sync.dma_start(out=rstd_b, in_=bnc_r[0:1, :].to_broadcast([P, QT]))
    for mi in range(DC):
        cen = lnP.tile([P, QT], f32, name=f"cen_{nm}", tag="cen")
        nc.vector.tensor_sub(cen, t[:, mi, :], mean_b)
        nrm = lnP.tile([P, QT], f32, name=f"nrm_{nm}", tag="nrm")
        nc.vector.tensor_mul(nrm, cen, rstd_b)
        nc.vector.tensor_scalar(
            out=out_f[:, mi, :],
            in0=nrm,
            scalar1=g_t[:, mi : mi + 1],
            scalar2=be_t[:, mi : mi + 1],
            op0=OP.mult,
            op1=OP.add,
        )
        if out_bf is not None:
            nc.vector.tensor_copy(out_bf[:, mi, :], out_f[:, mi, :])


def _get_nc():
    global _NC_CACHE
    if _NC_CACHE is None:
        _NC_CACHE = _build_nc()
    return _NC_CACHE


_RUNNER_CACHE = None


def _get_runner():
    """Cached jitted 8-core SPMD executor (avoids re-jitting per call).

    Modeled on bass2jax.run_bass_via_pjrt, but the jitted function and the
    sharding mesh are built once and reused.
    """
    global _RUNNER_CACHE
    if _RUNNER_CACHE is not None:
        return _RUNNER_CACHE

    import jax
    from jax.sharding import Mesh, PartitionSpec
    from jax.experimental.shard_map import shard_map
    from concourse import mybir as _mb
    from concourse.bass2jax import (
        _bass_exec_p,
        install_neuronx_cc_hook,
        partition_id_tensor,
    )

    install_neuronx_cc_hook()
    nc = _get_nc()
    n_cores = 8

    partition_name = nc.partition_id_tensor.name if nc.partition_id_tensor else None
    in_names = []
    out_names = []
    out_avals = []
    zero_outs = []
    for alloc in nc.m.functions[0].allocations:
        if not isinstance(alloc, _mb.MemoryLocationSet):
            continue
        name = alloc.memorylocations[0].name
        if alloc.kind == "ExternalInput":
            if name != partition_name:
                in_names.append(name)
        elif alloc.kind == "ExternalOutput":
            out_names.append(name)
            shape = tuple(alloc.tensor_shape)
            dtype = _mb.dt.np(alloc.dtype)
            out_avals.append(jax.core.ShapedArray(shape, dtype))
            zero_outs.append(np.zeros(shape, dtype))
    n_params = len(in_names)
    n_outs = len(out_avals)
    all_in_names = list(in_names) + list(out_names)
    if partition_name is not None:
        all_in_names.append(partition_name)

    def _body(*args):
        operands = list(args)
        if partition_name is not None:
            operands.append(partition_id_tensor())
        outs = _bass_exec_p.bind(
            *operands,
            out_avals=tuple(out_avals),
            in_names=tuple(all_in_names),
            out_names=tuple(out_names),
            lowering_input_output_aliases=(),
            sim_require_finite=True,
            sim_require_nnan=True,
            nc=nc,
        )
        return tuple(outs)

    devices = jax.devices()[:n_cores]
    mesh = Mesh(np.asarray(devices), ("core",))
    in_specs = (PartitionSpec("core"),) * (n_params + n_outs)
    out_specs = (PartitionSpec("core"),) * n_outs
    donate = tuple(range(n_params, n_params + n_outs))
    sharded = jax.jit(
        shard_map(
            _body, mesh=mesh, in_specs=in_specs, out_specs=out_specs, check_rep=False
        ),
        donate_argnums=donate,
        keep_unused=True,
    )

    _RUNNER_CACHE = {
        "fn": sharded,
        "in_names": in_names,
        "out_names": out_names,
        "out_avals": out_avals,
        "zero_outs": zero_outs,
        "n_cores": n_cores,
    }
    return _RUNNER_CACHE


def _run_spmd(in_maps):
    """Execute the kernel on 8 cores; returns list of per-core output dicts."""
    r = _get_runner()
    n_cores = r["n_cores"]
    concat_in = [
        np.concatenate([np.asarray(in_maps[c][name]) for c in range(n_cores)], axis=0)
        for name in r["in_names"]
    ]
    concat_zeros = [
        np.zeros((n_cores * z.shape[0], *z.shape[1:]), z.dtype) for z in r["zero_outs"]
    ]
    out_arrs = r["fn"](*concat_in, *concat_zeros)
    results = []
    for c in range(n_cores):
        results.append(
            {
                name: np.asarray(out_arrs[i]).reshape(
                    n_cores, *r["out_avals"][i].shape
                )[c]
                for i, name in enumerate(r["out_names"])
            }
        )
    return results


def _prep_shared(inputs):
    bf16 = ml_dtypes.bfloat16

    def f(a):
        return np.ascontiguousarray(a, dtype=np.float32)

    Wq, Wk, Wv, Wo = f(inputs["Wq"]), f(inputs["Wk"]), f(inputs["Wv"]), f(inputs["Wo"])
    W1, W2 = f(inputs["W1"]), f(inputs["W2"])
    shared = {
        "wq": np.ascontiguousarray(
            Wq.reshape(DC, P, DC, P).transpose(2, 1, 0, 3)
        ).astype(bf16),
        "wk": np.ascontiguousarray(
            Wk.reshape(DC, P, DC, P).transpose(2, 1, 0, 3)
        ).astype(bf16),
        "wv": np.ascontiguousarray(Wv.reshape(DC, P, D).transpose(1, 0, 2)).astype(
            bf16
        ),
        "wo": np.ascontiguousarray(
            Wo.reshape(DC, P, DC, P).transpose(2, 1, 0, 3)
        ).astype(bf16),
        "w1": np.ascontiguousarray(
            W1.reshape(DC, P, FC, P).transpose(2, 1, 0, 3)
        ).astype(bf16),
        "w2": np.ascontiguousarray(
            W2.reshape(FC, P, DC, P).transpose(2, 1, 0, 3)
        ).astype(bf16),
        "bq": np.ascontiguousarray(f(inputs["bq"]).reshape(DC, P).T),
        "bk": np.ascontiguousarray(f(inputs["bk"]).reshape(DC, P).T),
        "bvb": np.ascontiguousarray(np.broadcast_to(f(inputs["bv"]), (P, D))),
        "bo": np.ascontiguousarray(f(inputs["bo"]).reshape(DC, P).T),
        "b1": np.ascontiguousarray(f(inputs["b1"]).reshape(FC, P).T),
        "b2": np.ascontiguousarray(f(inputs["b2"]).reshape(DC, P).T),
        "g1": np.ascontiguousarray(f(inputs["g1"]).reshape(DC, P).T),
        "be1": np.ascontiguousarray(f(inputs["beta1"]).reshape(DC, P).T),
        "g2": np.ascontiguousarray(f(inputs["g2"]).reshape(DC, P).T),
        "be2": np.ascontiguousarray(f(inputs["beta2"]).reshape(DC, P).T),
    }
    return shared


def kernel(**inputs):
    bf16 = ml_dtypes.bfloat16
    x = np.ascontiguousarray(inputs["x"], dtype=np.float32)  # [2, 2048, 1024]
    B = x.shape[0]
    n_cores = 8
    per_batch = n_cores // B  # 4

    nc = _get_nc()
    shared = _prep_shared(inputs)

    in_maps = []
    xT_cache = {}
    for c in range(n_cores):
        b = c // per_batch
        qs = (c % per_batch) * QT
        if b not in xT_cache:
            xb = x[b]  # [S, D]
            xT_cache[b] = np.ascontiguousarray(
                xb.T.reshape(DC, P, S).transpose(1, 0, 2)
            )
        xTf = xT_cache[b]
        xq = x[b][qs : qs + QT]  # [QT, D]
        xqTf = np.ascontiguousarray(xq.T.reshape(DC, P, QT).transpose(1, 0, 2))
        m = dict(shared)
        m["xT"] = xTf.astype(bf16)
        m["xqT"] = xqTf.astype(bf16)
        m["xqTf"] = xqTf
        in_maps.append(m)

    results = _run_spmd(in_maps)

    out = np.empty((B, S, D), dtype=np.float32)
    for c in range(n_cores):
        b = c // per_batch
        qs = (c % per_batch) * QT
        oT = results[c]["oT"]  # [P, DC, QT]
        out[b, qs : qs + QT, :] = oT.transpose(2, 1, 0).reshape(QT, D)
    return out
